# revision 17
# baseline (speedup 1.0000x reference)
"""Trainium2 Bass kernel for nn_Bidir_Attention (top-k masked bidirectional
cross-attention) — fp16-split edition.

Data-parallel over batch: each of the 8 NeuronCores processes one batch
element end-to-end. W_qkv is replicated.

Score path: both directions factor through the same weight product, so the
host folds M' = 32 Wq Wk^T once (fp64) and the device computes
S1 = X1 M' X2^T / S2 = X2 M' X1^T in two GEMM stages — this removes the
separate K GEMM entirely (the "K^T" operand is X^T itself, which phase A
already produces for free).

Precision strategy (the top-16 selection needs fp32-class scores — one
rank-16/17 flip on a typical row costs ~5e-2 relative error, far over the
gate; fp32r measured at ~2^-13 is NOT usable there):
  - Both score stages run as 3-term fp16 splits (hi/lo), 3 cyc/row on the
    PE vs fp32's 4, each adding only ~2^-21 relative error (HW-probed:
    matches the 4-pass fp32 path). The 1/(32 sqrt(D)) scale is applied
    inside the exp activation so all split operands stay in fp16's healthy
    range.
  - V GEMM uses the fp16 hi parts only (~2^-11), AV runs in fp16 — both
    post-softmax, orders of magnitude inside the 2e-2 tolerance.

Self-contained: hardcodes B=8, N=2048, D=1024, topk=16.
"""

import sys

import numpy as np

for _p in ("/opt/trn_rl_repo", "/root/.axon_site/_ro/trn_rl_repo"):
    if _p not in sys.path:
        sys.path.append(_p)

import concourse.bacc as bacc
import concourse.mybir as mybir
from concourse.tile import TileContext
from concourse.masks import make_identity
from concourse.bass_utils import run_bass_kernel_spmd

B = 8
N = 2048
D = 1024
NT = N // 128          # 16 row tiles
DT = D // 128          # 8 contraction tiles
TOPK = 16
SCALE = float(1.0 / np.sqrt(D))
SC2 = float(SCALE / 32.0)     # scores computed against M' = 32 Wq Wk^T
NEG = -1e30
F32 = mybir.dt.float32
F16 = mybir.dt.float16
BF16 = mybir.dt.bfloat16


def _phase_a(nc, pools, x_dram, ident, mh, ml, wvh,
             tth_dram, ttl_dram, xth_dram, xtl_dram, v_dram):
    """Per feature: computes T^T = M'^T X^T (M' = 32 Wq Wk^T, folded on the
    host) and V = X Wv, and exports X^T itself — all as fp16 hi/lo pairs
    ([D,N]) except V ([N,D], fp16). The scores later become
    T1^T.T @ X2^T = X1 M' X2^T, saving the whole K GEMM."""
    sb, ps = pools
    for j in range(4):                      # supertiles of 512 rows
        xs = []
        for nsub in range(4):
            x = sb.tile([128, D], F32, tag=f"x{nsub}", bufs=2)
            nc.gpsimd.dma_start(
                out=x[:], in_=x_dram.ap()[j * 512 + nsub * 128: j * 512 + (nsub + 1) * 128, :])
            xs.append(x)
        xh = sb.tile([128, DT, 512], F16, tag="xh", bufs=2)
        xl = sb.tile([128, DT, 512], F16, tag="xl", bufs=2)
        for nsub in range(4):
            for di in range(DT):
                tp = ps.tile([128, 128], F32, tag="tp")
                nc.tensor.transpose(tp[:], xs[nsub][:, di * 128:(di + 1) * 128],
                                    ident[:])
                sl = (di, slice(nsub * 128, (nsub + 1) * 128))
                nc.vector.tensor_copy(xh[:, sl[0], sl[1]], tp[:])
                nc.vector.tensor_sub(xl[:, sl[0], sl[1]], tp[:], xh[:, sl[0], sl[1]])
        # export X^T (fp16 hi/lo) — it is the "K^T" of the other direction
        for di in range(DT):
            nc.gpsimd.dma_start(
                out=xth_dram.ap()[di * 128:(di + 1) * 128, j * 512:(j + 1) * 512],
                in_=xh[:, di, :])
            nc.gpsimd.dma_start(
                out=xtl_dram.ap()[di * 128:(di + 1) * 128, j * 512:(j + 1) * 512],
                in_=xl[:, di, :])
        # T^T = M'^T X^T: [dout 128-tile, n 512] pieces, fp16-split
        for t in range(8):
            qk_ps = ps.tile([128, 512], F32, tag="qk_ps")
            n_mm = DT * 3
            i = 0
            for di in range(DT):
                mhs = mh[di][t // 4][:, (t % 4) * 128:(t % 4 + 1) * 128]
                mls = ml[di][t // 4][:, (t % 4) * 128:(t % 4 + 1) * 128]
                for lhs, rhs in ((mhs, xh), (mhs, xl), (mls, xh)):
                    nc.tensor.matmul(qk_ps[:], lhs, rhs[:, di, :],
                                     start=(i == 0), stop=(i == n_mm - 1))
                    i += 1
            oh = sb.tile([128, 512], F16, tag="qkoh", bufs=2)
            ol = sb.tile([128, 512], F16, tag="qkol", bufs=2)
            nc.vector.tensor_copy(oh[:], qk_ps[:])
            nc.vector.tensor_sub(ol[:], qk_ps[:], oh[:])
            r0 = t * 128
            nc.gpsimd.dma_start(
                out=tth_dram.ap()[r0:r0 + 128, j * 512:(j + 1) * 512], in_=oh[:])
            nc.gpsimd.dma_start(
                out=ttl_dram.ap()[r0:r0 + 128, j * 512:(j + 1) * 512], in_=ol[:])
        # V: natural layout [n 128-tile, dout 512] pieces, fp16-hi only,
        # stored bf16
        for nsub in range(4):
            for c in range(2):
                v_ps = ps.tile([128, 512], F32, tag="v_ps")
                nsl = slice(nsub * 128, (nsub + 1) * 128)
                for di in range(DT):
                    nc.tensor.matmul(v_ps[:], xh[:, di, nsl], wvh[di][c][:],
                                     start=(di == 0), stop=(di == DT - 1))
                o = sb.tile([128, 512], F16, tag="vo", bufs=2)
                nc.vector.tensor_copy(o[:], v_ps[:])
                nc.gpsimd.dma_start(
                    out=v_dram.ap()[j * 512 + nsub * 128: j * 512 + (nsub + 1) * 128,
                                    c * 512:(c + 1) * 512],
                    in_=o[:])


def _phase_b(nc, pools, ident_h, qth_dram, qtl_dram, kth_dram, ktl_dram,
             v_dram, out_dram):
    """One attention direction: raw S = Q^T.T @ K^T via fp16 splits, softmax
    (scale folded into the exp), exact top-16 mask, bf16 AV, 1/Z renorm."""
    sbr, sb, ps = pools
    # residents: K^T hi/lo (fp16, 8MB) then V (bf16, 4MB), alternating
    # SP/SWDGE queues; ACT stays free for per-qi Q-tile prefetches.
    kth, ktl = [], []
    for di in range(DT):
        th = sbr.tile([128, N], F16, tag=f"kth{di}", name=f"kth{di}")
        nc.sync.dma_start(out=th[:], in_=kth_dram.ap()[di * 128:(di + 1) * 128, :])
        tl = sbr.tile([128, N], F16, tag=f"ktl{di}", name=f"ktl{di}")
        nc.gpsimd.dma_start(out=tl[:], in_=ktl_dram.ap()[di * 128:(di + 1) * 128, :])
        kth.append(th)
        ktl.append(tl)
    vres = []
    for nt in range(NT):
        v = sbr.tile([128, D], F16, tag=f"v{nt}", name=f"v{nt}")
        q = nc.sync if nt % 2 == 0 else nc.gpsimd
        q.dma_start(out=v[:], in_=v_dram.ap()[nt * 128:(nt + 1) * 128, :])
        vres.append(v)

    for qi in range(NT):
        qh, ql = [], []
        for di in range(DT):
            cs = slice(qi * 128, (qi + 1) * 128)
            th = sb.tile([128, 128], F16, tag=f"qth{di}", bufs=2)
            nc.scalar.dma_start(out=th[:], in_=qth_dram.ap()[di * 128:(di + 1) * 128, cs])
            tl = sb.tile([128, 128], F16, tag=f"qtl{di}", bufs=2)
            nc.scalar.dma_start(out=tl[:], in_=qtl_dram.ap()[di * 128:(di + 1) * 128, cs])
            qh.append(th)
            ql.append(tl)
        ssb = sb.tile([128, N], F32, tag="ssb", bufs=3)
        for half in range(2):
            s_ps = ps.tile([128, N // 2], F32, tag="s_ps", bufs=2)
            for c in range(2):
                ks = slice(half * 1024 + c * 512, half * 1024 + (c + 1) * 512)
                n_mm = DT * 3
                i = 0
                for di in range(DT):
                    for lhs, rhs in ((qh[di], kth[di]), (qh[di], ktl[di]),
                                     (ql[di], kth[di])):
                        nc.tensor.matmul(s_ps[:, c * 512:(c + 1) * 512],
                                         lhs[:], rhs[:, ks],
                                         start=(i == 0), stop=(i == n_mm - 1))
                        i += 1
            # split the PSUM->SBUF drain across both element engines
            nc.vector.tensor_copy(ssb[:, half * 1024:half * 1024 + 512],
                                  s_ps[:, 0:512])
            nc.scalar.copy(ssb[:, half * 1024 + 512:(half + 1) * 1024],
                           s_ps[:, 512:1024])

        m0 = sb.tile([128, 8], F32, tag="m0", bufs=3)
        nc.vector.max(out=m0[:], in_=ssb[:])
        nm = sb.tile([128, 1], F32, tag="nm", bufs=3)
        nc.vector.tensor_scalar_mul(nm[:], m0[:, 0:1], -SC2)
        p = sb.tile([128, N], F16, tag="p", bufs=2)
        z = sb.tile([128, 1], F32, tag="z", bufs=3)
        # p = exp((s - m0)/(32 sqrt(D))) — raw scores carry the extra 32 of
        # M' = 32 Wq Wk^T; both scales fold into the exp here
        nc.scalar.activation(p[:], ssb[:], mybir.ActivationFunctionType.Exp,
                             bias=nm[:], scale=SC2, accum_out=z[:])
        iz = sb.tile([128, 1], F32, tag="iz", bufs=3)
        nc.vector.reciprocal(iz[:], z[:])
        # exact top-16: two rounds of max8 + match_replace (in place on ssb,
        # which the Exp above has already consumed)
        nc.vector.match_replace(out=ssb[:], in_to_replace=m0[:], in_values=ssb[:],
                                imm_value=NEG)
        m8 = sb.tile([128, 8], F32, tag="m8", bufs=3)
        nc.vector.max(out=m8[:], in_=ssb[:])
        nc.vector.match_replace(out=ssb[:], in_to_replace=m8[:], in_values=ssb[:],
                                imm_value=NEG)
        # A = exp-weights where selected else 0   (in place on p)
        nc.vector.scalar_tensor_tensor(out=p[:], in0=ssb[:], scalar=NEG, in1=p[:],
                                       op0=mybir.AluOpType.is_equal,
                                       op1=mybir.AluOpType.mult)
        # transpose A tiles for the AV matmul; AV runs in fp16 (1 cyc/row
        # transposes, ~2^-11 weight rounding)
        ats = []
        for kt_i in range(NT):
            tp = ps.tile([128, 128], F16, tag="tp2")
            nc.tensor.transpose(tp[:], p[:, kt_i * 128:(kt_i + 1) * 128], ident_h[:])
            at = sb.tile([128, 128], F16, tag=f"at{kt_i}", bufs=2)
            if kt_i % 2:
                nc.vector.tensor_copy(at[:], tp[:])
            else:
                nc.scalar.copy(at[:], tp[:])
            ats.append(at)
        osb = sb.tile([128, D], F32, tag="osb")
        for h in range(2):
            o_ps = ps.tile([128, 512], F32, tag="o_ps")
            hs = slice(h * 512, (h + 1) * 512)
            for kt_i in range(NT):
                nc.tensor.matmul(o_ps[:], ats[kt_i][:], vres[kt_i][:, hs],
                                 start=(kt_i == 0), stop=(kt_i == NT - 1))
            nc.scalar.mul(osb[:, hs], o_ps[:], iz[:])
        nc.gpsimd.dma_start(out=out_dram.ap()[qi * 128:(qi + 1) * 128, :], in_=osb[:])


def build():
    nc = bacc.Bacc()
    f1 = nc.declare_dram_parameter("feature1", [N, D], F32, isOutput=False)
    f2 = nc.declare_dram_parameter("feature2", [N, D], F32, isOutput=False)
    # The weight arrives pre-folded on the host (see kernel()): M' =
    # 32 Wq Wk^T as fp16 hi/lo, W_v as fp16 hi.
    mh_d = nc.declare_dram_parameter("mh", [D, D], F16, isOutput=False)
    ml_d = nc.declare_dram_parameter("ml", [D, D], F16, isOutput=False)
    wvh_d = nc.declare_dram_parameter("wvh", [D, D], F16, isOutput=False)
    out1 = nc.declare_dram_parameter("out1", [N, D], F32, isOutput=True)
    out2 = nc.declare_dram_parameter("out2", [N, D], F32, isOutput=True)

    scr = {}
    for f in (1, 2):
        for nm in ("tth", "ttl", "xth", "xtl"):
            scr[f"{nm}{f}"] = nc.dram_tensor(f"{nm}{f}", [D, N], F16)
        scr[f"v{f}"] = nc.dram_tensor(f"v{f}", [N, D], F16)

    with TileContext(nc) as tc:
        with tc.tile_pool(name="const", bufs=1) as constp:
            ident = constp.tile([128, 128], F32, tag="id_f")
            make_identity(nc, ident[:])
            ident_h = constp.tile([128, 128], F16, tag="id_h")
            nc.vector.tensor_copy(ident_h[:], ident[:])

            with (
                tc.tile_pool(name="wpool", bufs=1) as wp,
                tc.tile_pool(name="apool", bufs=1) as asb,
                tc.tile_pool(name="apsum", bufs=2, space="PSUM") as aps,
            ):
                # Weight loads: pre-folded fp16, 512-wide column blocks in
                # consumption order (M' hi/lo pairs then V hi), alternating
                # between the two HWDGE queues.
                mh = [[None] * 2 for _ in range(DT)]
                ml = [[None] * 2 for _ in range(DT)]
                wvh = [[None] * 2 for _ in range(DT)]
                for tb in range(2):
                    for di in range(DT):
                        cs = slice(tb * 512, (tb + 1) * 512)
                        h = wp.tile([128, 512], F16, tag=f"mh{di}_{tb}")
                        nc.scalar.dma_start(
                            out=h[:], in_=mh_d.ap()[di * 128:(di + 1) * 128, cs])
                        l = wp.tile([128, 512], F16, tag=f"mlo{di}_{tb}")
                        nc.sync.dma_start(
                            out=l[:], in_=ml_d.ap()[di * 128:(di + 1) * 128, cs])
                        mh[di][tb] = h
                        ml[di][tb] = l
                for c in range(2):
                    for di in range(DT):
                        h = wp.tile([128, 512], F16, tag=f"wvh{di}_{c}")
                        q = nc.scalar if di % 2 else nc.sync
                        q.dma_start(
                            out=h[:],
                            in_=wvh_d.ap()[di * 128:(di + 1) * 128,
                                           c * 512:(c + 1) * 512])
                        wvh[di][c] = h
                with nc.named_scope("phaseA_f1"):
                    _phase_a(nc, (asb, aps), f1, ident, mh, ml, wvh,
                             scr["tth1"], scr["ttl1"], scr["xth1"], scr["xtl1"],
                             scr["v1"])
                with nc.named_scope("phaseA_f2"):
                    _phase_a(nc, (asb, aps), f2, ident, mh, ml, wvh,
                             scr["tth2"], scr["ttl2"], scr["xth2"], scr["xtl2"],
                             scr["v2"])

            with (
                tc.tile_pool(name="bpool", bufs=1) as bsb,
                tc.tile_pool(name="bwork", bufs=2) as bwk,
                tc.tile_pool(name="bpsum", bufs=2, space="PSUM") as bps,
            ):
                with nc.named_scope("phaseB_d1"):
                    _phase_b(nc, (bsb, bwk, bps), ident_h,
                             scr["tth1"], scr["ttl1"], scr["xth2"], scr["xtl2"],
                             scr["v2"], out1)
                with nc.named_scope("phaseB_d2"):
                    _phase_b(nc, (bsb, bwk, bps), ident_h,
                             scr["tth2"], scr["ttl2"], scr["xth1"], scr["xtl1"],
                             scr["v1"], out2)
    return nc


_NC_CACHE = None


def _get_nc():
    global _NC_CACHE
    if _NC_CACHE is None:
        _NC_CACHE = build()
        _NC_CACHE.finalize()
    return _NC_CACHE


def split_w(w):
    """Host-side one-time transform of the replicated weight: fold the
    bilinear score form M' = 32 Wq Wk^T (computed in fp64), split to fp16
    hi/lo; W_v as fp16 hi."""
    wq = w[:, :D].astype(np.float64)
    wk = w[:, D:2 * D].astype(np.float64)
    m = (32.0 * (wq @ wk.T)).astype(np.float32)
    mh = m.astype(np.float16)
    ml = (m - mh.astype(np.float32)).astype(np.float16)
    wvh = np.ascontiguousarray(w[:, 2 * D:]).astype(np.float16)
    return mh, ml, wvh


def kernel(feature1, feature2, W_qkv, topk):
    assert int(topk) == TOPK, f"kernel hardcodes topk=16, got {topk}"
    f1 = np.ascontiguousarray(np.asarray(feature1), dtype=np.float32)
    f2 = np.ascontiguousarray(np.asarray(feature2), dtype=np.float32)
    w = np.ascontiguousarray(np.asarray(W_qkv), dtype=np.float32)
    assert f1.shape == (B, N, D) and f2.shape == (B, N, D) and w.shape == (D, 3 * D)

    nc = _get_nc()
    mh, ml, wvh = split_w(w)
    in_maps = [{"feature1": f1[b], "feature2": f2[b],
                "mh": mh, "ml": ml, "wvh": wvh} for b in range(B)]
    try:
        res = run_bass_kernel_spmd(nc, in_maps, list(range(B))).results
    except Exception:
        # transient device faults have been observed; one retry on a fresh
        # execution usually clears them
        res = run_bass_kernel_spmd(nc, in_maps, list(range(B))).results
    o1 = np.stack([res[b]["out1"] for b in range(B)]).astype(np.float32)
    o2 = np.stack([res[b]["out2"] for b in range(B)]).astype(np.float32)
    return o1, o2


if __name__ == "__main__":
    f1 = np.load("/root/problem/cache/f1.npy")
    f2 = np.load("/root/problem/cache/f2.npy")
    w = np.load("/root/problem/cache/W.npy")
    o1, o2 = kernel(f1, f2, w, 16)
    r1 = np.load("/root/problem/cache/r1.npy")
    r2 = np.load("/root/problem/cache/r2.npy")
    for nm, o, r in (("2to1", o1, r1), ("1to2", o2, r2)):
        err = np.abs(o - r).max()
        rel = err / np.abs(r).max()
        print(f"{nm}: absmax_err={err:.3e} rel={rel:.3e}")
